# revision 7
# baseline (speedup 1.0000x reference)
"""Dense transformer block (LN1 -> causal MHA -> proj -> residual -> LN2 ->
FC1 -> gelu(tanh) -> FC2 -> residual) on 8 Trainium2 NeuronCores.

Sharding: two SPMD launches, no on-device collectives.
  Launch 1 (tensor-parallel over batch x head-group): core c = (batch c//4,
    heads 4*(c%4)..4*(c%4)+3). LayerNorm1 is folded into the QKV matmul via
    host-precomputed W*g weights plus augmented contraction rows carrying the
    per-row mean and std. Q/K projections run as fp8e4 DoubleRowSwInterleave
    matmuls (2 K-tiles per instruction) against an fp8 copy of x^T; the V
    projection and attention (scores^T = K @ Q^T with a ones-column in V
    accumulating the softmax denominator) stay bf16 for accuracy. The output
    projection runs fp8 DR over head pairs producing partial proj^T [D, S];
    the host scales by 2^-11, transposes, sums the 4 partials per batch and
    adds the residual. All fp8 weights carry a global 2^11 scale; the
    1/std row is pre-multiplied by 2^-11 so dequantization is free.
  Launch 2 (data-parallel over tokens): core c takes 512 of the 4096 rows of
    h' and computes LN2 + FC1 + gelu + FC2 in bf16. FC2 uses gelu outputs as
    the stationary operand (halves weight reloads) and emits natural-layout
    [rows, D] partial sums; the host adds the residual.
"""

import sys

if "/opt/trn_rl_repo" not in sys.path:
    sys.path.insert(0, "/opt/trn_rl_repo")

import numpy as np
import ml_dtypes

import concourse.bass as bass
import concourse.tile as tile
from concourse import mybir
import bass_rust
from concourse.bass_utils import run_bass_kernel_spmd

B, S, D, H, DH, DFF = 2, 2048, 1024, 16, 64, 4096
NCORES = 8
HG = 4          # heads per core
QC = 512        # query chunk
KB = 128        # key block
NQ = S // QC    # 4 query chunks
NRT = S // 128  # 16 row tiles
EPS = 1e-5
ROWS2 = (B * S) // NCORES  # 512 rows per core in launch 2
SW = 2048.0     # global fp8 weight scale (2^11)
ISW = 1.0 / SW

bf16 = mybir.dt.bfloat16
f8 = mybir.dt.float8e4
f32 = mybir.dt.float32
nbf = ml_dtypes.bfloat16
nf8 = ml_dtypes.float8_e4m3

AF = mybir.ActivationFunctionType
ALU = mybir.AluOpType
DRSWI = mybir.MatmulPerfMode.DoubleRowSwInterleave


def _finish(nc):
    bass_rust.move_matmul_waits_to_ldweights(nc.m)
    bass_rust.generate_event_semaphores(nc)
    return nc


def _swi(A, Bm):
    """SwInterleave stationary layout: per-partition A_{M-1} B_{M-1} ... A_0 B_0."""
    K, M = A.shape
    o = np.empty((K, 2 * M), np.float32)
    o[:, 0::2] = A[:, ::-1]
    o[:, 1::2] = Bm[:, ::-1]
    return o


# --------------------------------------------------------------------------
# Launch 1: LN1 + QKV + causal attention + partial proj
# --------------------------------------------------------------------------
def build_l1(rep=1):
    nc = bass.Bass()
    d_x8 = nc.declare_dram_parameter("x8", [D, S], f8, isOutput=False)
    d_xT = nc.declare_dram_parameter("xT", [D, S], bf16, isOutput=False)
    d_wqk = nc.declare_dram_parameter("wqk", [4, 4, 128, 256], f8, isOutput=False)
    d_wqka = nc.declare_dram_parameter("wqka", [1, 1024], bf16, isOutput=False)
    d_wv = nc.declare_dram_parameter("wv", [D, 256], bf16, isOutput=False)
    d_uv = nc.declare_dram_parameter("uv", [1, 256], bf16, isOutput=False)
    d_wp = nc.declare_dram_parameter("wp", [8, 2, 64, 256], f8, isOutput=False)
    d_masks = nc.declare_dram_parameter("masks", [4, KB, QC], bf16, isOutput=False)
    d_out = nc.declare_dram_parameter("out", [D, S], f32, isOutput=True)

    with tile.TileContext(nc) as tc:
        with (
            tc.tile_pool(name="const", bufs=1) as const,
            tc.tile_pool(name="persist", bufs=1) as persist,
            tc.tile_pool(name="exps", bufs=6) as exps,
            tc.tile_pool(name="evict", bufs=3) as evict,
            tc.tile_pool(name="pbig", bufs=3, space="PSUM") as pbig,
            tc.tile_pool(name="pacc", bufs=1, space="PSUM") as pacc,
            tc.tile_pool(name="py", bufs=2, space="PSUM") as py,
            tc.tile_pool(name="ptiny", bufs=1, space="PSUM") as ptiny,
        ):
            # ---- constants / persistent tiles ----
            t_wqk = const.tile([128, 4, 4, 2, 128], f8)
            t_wv = const.tile([128, 8, 256], bf16)
            t_wp = const.tile([64, 8, 2, 2, 128], f8)
            t_wqka = const.tile([1, 1024], bf16)
            t_uv = const.tile([1, 256], bf16)
            t_masks = const.tile([128, 4, QC], bf16)
            t_ones = const.tile([128, 128], bf16)
            t_eps = const.tile([1, 1], f32)
            nc.sync.dma_start(t_wqka[:], d_wqka[:])
            nc.sync.dma_start(t_uv[:], d_uv[:])
            nc.sync.dma_start(t_masks[:], d_masks[:].rearrange("j p n -> p j n"))
            for ct in range(4):
                for j in range(4):
                    nc.sync.dma_start(
                        t_wqk[:, ct, j].rearrange("p a m -> p (a m)"),
                        d_wqk[ct, j])
            for ci in range(8):
                nc.sync.dma_start(t_wv[:, ci, :], d_wv[ci * 128:(ci + 1) * 128, :])
            for dc in range(8):
                for pp in range(2):
                    nc.sync.dma_start(
                        t_wp[:, dc, pp].rearrange("p a m -> p (a m)"),
                        d_wp[dc, pp])
            nc.vector.memset(t_ones[:], 1.0)
            nc.vector.memset(t_eps[:], EPS * (SW * SW))

            t_x8 = persist.tile([128, 8, S], f8)
            t_xT = persist.tile([128, 8, S], bf16)
            for ci in range(8):
                nc.sync.dma_start(t_x8[:, ci, :], d_x8[ci * 128:(ci + 1) * 128, :])
                nc.sync.dma_start(t_xT[:, ci, :], d_xT[ci * 128:(ci + 1) * 128, :])

            # Q01 / K01 / Q23 / K23 transposed pair tiles [128, S]
            t_qk = [persist.tile([128, S], bf16, tag=f"qk{i}", name=f"qk{i}")
                    for i in range(4)]
            # V natural, per row-tile: [128, head, 65] (col 64 = ones)
            t_vau = persist.tile([128, NRT, HG, 65], bf16)
            # Y^T per head, fp8 pairs for the proj DR: [64, 4h, S]
            t_y = persist.tile([64, 4, S], f8)
            # per-row LN rows
            t_negmu = persist.tile([1, S], bf16)
            t_std = persist.tile([1, S], bf16)
            t_rstd = persist.tile([1, S], bf16)      # 2^-11/std

            for _r in range(rep):
              # ---- LN stats for all row chunks up front ----
              for qi in range(NQ):
                rsl = bass.ts(qi, QC)
                p_sx = ptiny.tile([1, QC], f32, tag="sx")
                p_sx2 = ptiny.tile([1, QC], f32, tag="sx2")
                for ci in range(8):
                    nc.tensor.matmul(p_sx[:], t_ones[:, 0:1],
                                     t_xT[:, ci, rsl],
                                     start=(ci == 0), stop=(ci == 7),
                                     skip_group_check=True)
                for ci in range(8):
                    sq = evict.tile([128, QC], bf16, tag="sq")
                    nc.vector.tensor_mul(out=sq[:], in0=t_xT[:, ci, rsl],
                                         in1=t_xT[:, ci, rsl])
                    nc.tensor.matmul(p_sx2[:], t_ones[:, 0:1],
                                     sq[:],
                                     start=(ci == 0), stop=(ci == 7),
                                     skip_group_check=True)
                # mu, var, 2^11*std, 2^-11/std rows
                mu_f = evict.tile([1, QC], f32, tag="mu")
                m2_f = evict.tile([1, QC], f32, tag="m2")
                nc.scalar.activation(mu_f[:], p_sx[:], AF.Copy, scale=1.0 / D)
                nc.scalar.activation(m2_f[:], p_sx2[:], AF.Copy, scale=1.0 / D)
                nc.scalar.activation(t_negmu[0:1, rsl], p_sx[:], AF.Copy, scale=-1.0 / D)
                var_f = evict.tile([1, QC], f32, tag="var")
                nc.vector.tensor_mul(out=mu_f[:], in0=mu_f[:], in1=mu_f[:])
                nc.vector.tensor_tensor(out=var_f[:], in0=m2_f[:], in1=mu_f[:],
                                        op=ALU.subtract)
                std_f = evict.tile([1, QC], f32, tag="std")
                # std_f = 2^11 * std  (Sqrt of 2^22*(var+eps))
                nc.scalar.activation(std_f[:], var_f[:], AF.Sqrt, bias=t_eps[:],
                                     scale=SW * SW)
                nc.scalar.activation(t_std[0:1, rsl], std_f[:], AF.Copy, scale=ISW)
                with nc.allow_low_precision(reason="rstd feeds bf16 matmul"):
                    nc.vector.reciprocal(out=t_rstd[0:1, rsl], in_=std_f[:])

              for qi in range(NQ):
                rsl = bass.ts(qi, QC)
                # ================= phase A: QKV^T + V =================
                # broadcast 2^-11/std over 128 partitions
                p_bc = pacc.tile([128, QC], f32, tag="acc")
                nc.tensor.matmul(p_bc[:], t_ones[0:1, :], t_rstd[0:1, rsl],
                                 start=True, stop=True)
                rstd_b = evict.tile([128, QC], f32, tag="rstdb")
                nc.vector.tensor_copy(rstd_b[:], p_bc[:])

                # QKV^T for Q/K: 4 column tiles, fp8 DR over ci pairs
                for ct in range(4):
                    p_qk = pbig.tile([128, QC], f32, tag="big")
                    for j in range(4):
                        nc.tensor.matmul(p_qk[:], t_wqk[:, ct, j],
                                         t_x8[:, 2 * j:2 * j + 2, rsl],
                                         start=(j == 0), stop=False,
                                         perf_mode=DRSWI)
                    nc.tensor.matmul(p_qk[:], t_wqka[0:1, bass.ts(ct, 128)],
                                     t_negmu[0:1, rsl],
                                     start=False, stop=False, skip_group_check=True)
                    nc.tensor.matmul(p_qk[:],
                                     t_wqka[0:1, bass.ds(512 + ct * 128, 128)],
                                     t_std[0:1, rsl],
                                     start=False, stop=True, skip_group_check=True)
                    nc.vector.tensor_mul(out=t_qk[ct][:, rsl], in0=p_qk[:],
                                         in1=rstd_b[:])

                # V natural for the 4 row tiles of this chunk (bf16)
                for rt in range(qi * 4, qi * 4 + 4):
                    rtsl = bass.ts(rt, 128)
                    p_v = pacc.tile([128, 256], f32, tag="acc")
                    for ci in range(8):
                        nc.tensor.matmul(p_v[:], t_xT[:, ci, rtsl],
                                         t_wv[:, ci, :],
                                         start=(ci == 0), stop=False)
                    nc.tensor.matmul(p_v[:], t_negmu[0:1, rtsl], t_uv[:],
                                     start=False, stop=True, skip_group_check=True)
                    # 2^-11/std as a natural [128,1] column via K=1 matmul
                    p_t = ptiny.tile([128, 1], f32, tag="sx")
                    nc.tensor.matmul(p_t[:], t_rstd[0:1, rtsl], t_ones[0:1, 0:1],
                                     start=True, stop=True)
                    rstd_n = evict.tile([128, 1], f32, tag="rstdn")
                    nc.vector.tensor_copy(rstd_n[:], p_t[:])
                    nc.vector.tensor_scalar_mul(
                        out=t_vau[:, rt, :, 0:64],
                        in0=p_v[:].rearrange("p (h n) -> p h n", h=HG),
                        scalar1=rstd_n[:])
                    nc.vector.memset(t_vau[:, rt, :, 64:65], 1.0)

                # ================= phase B: attention =================
                for pr in range(2):
                    qt = t_qk[2 * pr]
                    kt = t_qk[2 * pr + 1]
                    nkb = 4 * (qi + 1)
                    hs = [2 * pr, 2 * pr + 1]
                    p_ys = {}
                    for h in hs:
                        p_ys[h] = py.tile([65, QC], f32, tag="y", name=f"py{h}")
                    for kb in range(nkb):
                        diag = kb >= 4 * qi
                        jj = kb - 4 * qi
                        qo = 128 * jj if diag else 0
                        qn = QC - qo
                        es = {}
                        for h in hs:
                            off = 64 * (h % 2)
                            p_s = pbig.tile([128, QC], f32, tag="big",
                                            name=f"ps{h}")
                            nc.tensor.matmul(
                                p_s[:, qo:QC],
                                kt[off:off + 64, bass.ts(kb, KB)],
                                qt[off:off + 64, bass.ds(qi * QC + qo, qn)],
                                start=True, stop=True,
                                skip_group_check=True)
                            e = exps.tile([128, QC], bf16, tag="e",
                                          name=f"e{h}")
                            nc.scalar.activation(e[:, qo:QC], p_s[:, qo:QC],
                                                 AF.Exp)
                            if diag:
                                nc.vector.tensor_mul(
                                    out=e[:, qo:QC], in0=e[:, qo:QC],
                                    in1=t_masks[:, jj, qo:QC])
                            es[h] = e
                        for h in hs:
                            nc.tensor.matmul(
                                p_ys[h][:, qo:QC],
                                t_vau[:, kb, h, :],
                                es[h][:, qo:QC],
                                start=(kb == 0), stop=(kb == nkb - 1),
                                skip_group_check=True)
                    # normalize: Y[0:64] * broadcast(1/se) -> fp8 for proj
                    for h in hs:
                        p_y = p_ys[h]
                        se = evict.tile([65, QC], bf16, tag="se")
                        with nc.allow_low_precision(reason="softmax denom feeds fp8 matmul"):
                            nc.vector.reciprocal(out=se[64:65, :], in_=p_y[64:65, :])
                        p_n = pacc.tile([64, QC], f32, tag="acc")
                        nc.tensor.matmul(p_n[:], t_ones[64:65, 0:64], se[64:65, :],
                                         start=True, stop=True)
                        bc = evict.tile([64, QC], f32, tag="bc")
                        nc.vector.tensor_copy(bc[:], p_n[:])
                        with nc.allow_low_precision(reason="y feeds fp8 matmul"):
                            nc.vector.tensor_mul(out=t_y[:, h, rsl],
                                                 in0=p_y[0:64, :], in1=bc[:])

                # ===== phase C: partial proj^T via fp8 DR head pairs =====
                for dc in range(8):
                    p_o = pbig.tile([128, QC], f32, tag="big")
                    for pp in range(2):
                        nc.tensor.matmul(p_o[:], t_wp[:, dc, pp],
                                         t_y[:, 2 * pp:2 * pp + 2, rsl],
                                         start=(pp == 0), stop=(pp == 1),
                                         perf_mode=DRSWI)
                    o_sb = evict.tile([128, QC], f32, tag="osb")
                    nc.vector.tensor_copy(o_sb[:], p_o[:])
                    nc.sync.dma_start(d_out[bass.ts(dc, 128), rsl], o_sb[:])

    return _finish(nc)


# --------------------------------------------------------------------------
# Launch 2: LN2 + FC1 + gelu + FC2 (rows sharded), bf16
# --------------------------------------------------------------------------
def build_l2(rep=1):
    R = ROWS2
    nc = bass.Bass()
    d_hT = nc.declare_dram_parameter("hT", [D, R], bf16, isOutput=False)
    d_wfc = nc.declare_dram_parameter("wfc", [32, 8, 128, 128], bf16, isOutput=False)
    d_cfc = nc.declare_dram_parameter("cfc", [128, 32], f32, isOutput=False)
    d_w2 = nc.declare_dram_parameter("w2", [32, 128, 1024], bf16, isOutput=False)
    d_b2r = nc.declare_dram_parameter("b2r", [128, 1024], f32, isOutput=False)
    d_out = nc.declare_dram_parameter("out", [R, D], f32, isOutput=True)

    with tile.TileContext(nc) as tc:
        with (
            tc.tile_pool(name="const", bufs=1) as const,
            tc.tile_pool(name="persist", bufs=1) as persist,
            tc.tile_pool(name="evict", bufs=2) as evict,
            tc.tile_pool(name="osbp", bufs=2) as osbp,
            tc.tile_pool(name="pbig", bufs=4, space="PSUM") as pbig,
            tc.tile_pool(name="pacc", bufs=1, space="PSUM") as pacc,
            tc.tile_pool(name="ptiny", bufs=1, space="PSUM") as ptiny,
        ):
            t_cfc = const.tile([128, 32], f32)
            t_b2r = const.tile([128, 1024], f32)
            t_ones = const.tile([128, 1], bf16)
            t_onesr = const.tile([1, 128], f32)
            t_eps = const.tile([1, 1], f32)
            nc.sync.dma_start(t_cfc[:], d_cfc[:])
            nc.sync.dma_start(t_b2r[:], d_b2r[:])
            nc.vector.memset(t_ones[:], 1.0)
            nc.vector.memset(t_onesr[:], 1.0)
            nc.vector.memset(t_eps[:], EPS)

            t_hT = persist.tile([128, 8, R], bf16)
            for ci in range(8):
                nc.sync.dma_start(t_hT[:, ci, :], d_hT[ci * 128:(ci + 1) * 128, :])

            t_wfc = persist.tile([128, 32, 8, 128], bf16)
            for ct in range(32):
                nc.sync.dma_start(t_wfc[:, ct, :, :],
                                  d_wfc[ct].rearrange("c p n -> p c n"))
            t_w2 = persist.tile([128, 32, 1024], bf16)
            for ci in range(32):
                nc.sync.dma_start(t_w2[:, ci, :], d_w2[ci])

            t_h1 = persist.tile([128, 32, R], bf16)  # gelu outputs, transposed
            t_aug = persist.tile([1, R], f32)        # -mu row
            t_rstd = persist.tile([1, R], f32)

            for _r in range(rep):
              # ---- stats ----
              p_sx = ptiny.tile([1, R], f32, tag="sx")
              p_sx2 = ptiny.tile([1, R], f32, tag="sx2")
              for ci in range(8):
                  nc.tensor.matmul(p_sx[:], t_ones[:, 0:1],
                                   t_hT[:, ci, :], start=(ci == 0), stop=(ci == 7),
                                   skip_group_check=True)
              for ci in range(8):
                  sq = evict.tile([128, R], bf16, tag="sq")
                  nc.scalar.activation(sq[:], t_hT[:, ci, :], AF.Square)
                  nc.tensor.matmul(p_sx2[:], t_ones[:, 0:1],
                                   sq[:], start=(ci == 0), stop=(ci == 7),
                                   skip_group_check=True)
              mu_f = evict.tile([1, R], f32, tag="mu")
              m2_f = evict.tile([1, R], f32, tag="m2")
              nc.scalar.activation(mu_f[:], p_sx[:], AF.Copy, scale=1.0 / D)
              nc.scalar.activation(m2_f[:], p_sx2[:], AF.Copy, scale=1.0 / D)
              nc.scalar.activation(t_aug[0:1, :], p_sx[:], AF.Copy, scale=-1.0 / D)
              var_f = evict.tile([1, R], f32, tag="var")
              nc.vector.tensor_mul(out=mu_f[:], in0=mu_f[:], in1=mu_f[:])
              nc.vector.tensor_tensor(out=var_f[:], in0=m2_f[:], in1=mu_f[:],
                                      op=ALU.subtract)
              std_f = evict.tile([1, R], f32, tag="std")
              nc.scalar.activation(std_f[:], var_f[:], AF.Sqrt, bias=t_eps[:])
              nc.vector.reciprocal(out=t_rstd[0:1, :], in_=std_f[:])

              # ---- broadcast -mu and rstd over partitions (PE rank-1) ----
              p_nm = pacc.tile([128, R], f32, tag="nm")
              p_rs = pacc.tile([128, R], f32, tag="rs")
              nc.tensor.matmul(p_nm[:], t_onesr[:], t_aug[0:1, :],
                               start=True, stop=True)
              nc.tensor.matmul(p_rs[:], t_onesr[:], t_rstd[0:1, :],
                               start=True, stop=True)

              # ---- normalize in place: hT <- (h - mu) * rstd ----
              # (rep>1 timing replicas then run on clobbered values; the
              # instruction stream and timing are identical, and the real
              # kernel() dispatch uses rep=1.)
              for ci in range(8):
                  cen = evict.tile([128, R], f32, tag="cen")
                  nc.vector.tensor_tensor(out=cen[:], in0=t_hT[:, ci, :],
                                          in1=p_nm[:], op=ALU.add)
                  nc.vector.tensor_mul(out=t_hT[:, ci, :], in0=cen[:],
                                       in1=p_rs[:])

              # ---- FC1 + gelu ----
              for ct in range(32):
                  p1 = pbig.tile([128, R], f32, tag="big")
                  for ci in range(8):
                      nc.tensor.matmul(p1[:], t_wfc[:, ct, ci, :], t_hT[:, ci, :],
                                       start=(ci == 0), stop=(ci == 7))
                  nc.scalar.activation(t_h1[:, ct, :], p1[:], AF.Gelu_apprx_tanh,
                                       bias=t_cfc[:, ct:ct + 1])

              # ---- FC2: stationary = h1 row-tiles, natural [rows, D] out ----
              for rt in range(4):
                  rtsl = bass.ts(rt, 128)
                  p2a = pbig.tile([128, 512], f32, tag="big")
                  p2b = pbig.tile([128, 512], f32, tag="big")
                  for ci in range(32):
                      nc.tensor.matmul(p2a[:], t_h1[:, ci, rtsl],
                                       t_w2[:, ci, 0:512],
                                       start=(ci == 0), stop=(ci == 31),
                                       skip_group_check=True)
                      nc.tensor.matmul(p2b[:], t_h1[:, ci, rtsl],
                                       t_w2[:, ci, 512:1024],
                                       start=(ci == 0), stop=(ci == 31),
                                       skip_group_check=True)
                  o_sb = osbp.tile([128, 1024], f32, tag="osb")
                  nc.vector.tensor_tensor(out=o_sb[:, 0:512], in0=p2a[:],
                                          in1=t_b2r[:, 0:512], op=ALU.add)
                  nc.vector.tensor_tensor(out=o_sb[:, 512:1024], in0=p2b[:],
                                          in1=t_b2r[:, 512:1024], op=ALU.add)
                  nc.sync.dma_start(d_out[rtsl, :], o_sb[:])

    return _finish(nc)


# --------------------------------------------------------------------------
# Host glue
# --------------------------------------------------------------------------
_CACHE = {}


def _get_l1():
    if "l1" not in _CACHE:
        _CACHE["l1"] = build_l1()
    return _CACHE["l1"]


def _get_l2():
    if "l2" not in _CACHE:
        _CACHE["l2"] = build_l2()
    return _CACHE["l2"]


def _make_masks():
    k = np.arange(KB)[:, None]
    q = np.arange(QC)[None, :]
    m = np.zeros((4, KB, QC), np.float32)
    for jj in range(4):
        m[jj] = np.where(128 * jj + k <= q, 1.0, 0.0)
    return m.astype(nbf)


def prep_l1_inputs(hidden_states, ln1_g, ln1_b, w_attn, b_attn):
    h = np.asarray(hidden_states, np.float32)
    g1 = np.asarray(ln1_g, np.float32)
    b1 = np.asarray(ln1_b, np.float32)
    wa = np.asarray(w_attn, np.float32)
    ba = np.asarray(b_attn, np.float32)
    wg = wa * g1[:, None]
    const_all = b1 @ wa + ba  # [3D]
    masks = _make_masks()
    xT = [np.ascontiguousarray(h[b].T) for b in range(B)]
    xT_bf = [x.astype(nbf) for x in xT]
    xT_f8 = [x.astype(nf8) for x in xT]
    sc = 1.0 / np.sqrt(DH)

    in_maps = []
    for c in range(NCORES):
        b, g = c // HG, c % HG
        heads = [HG * g + i for i in range(HG)]
        qcols, kcols, vcols = [], [], []
        for hh in heads:
            qcols += list(range(DH * hh, DH * hh + DH))
            kcols += list(range(D + DH * hh, D + DH * hh + DH))
            vcols += list(range(2 * D + DH * hh, 2 * D + DH * hh + DH))
        # wqk col order: Q01 | K01 | Q23 | K23
        cols = (qcols[:128] + kcols[:128] + qcols[128:] + kcols[128:])
        scale = np.array([sc] * 128 + [1.0] * 128 + [sc] * 128 + [1.0] * 128,
                         np.float32)
        wqk_full = (wg[:, cols] * scale[None, :] * SW).astype(nf8).astype(
            np.float32)                                   # [D, 512] quantized
        # [4ct, 4j, 128, 256] SwInterleaved ci pairs
        wqk = np.empty((4, 4, 128, 256), nf8)
        for ct in range(4):
            wcol = wqk_full[:, ct * 128:(ct + 1) * 128]
            for j in range(4):
                A = wcol[256 * j:256 * j + 128, :]
                Bm = wcol[256 * j + 128:256 * j + 256, :]
                wqk[ct, j] = _swi(A, Bm).astype(nf8)
        u = wqk_full.sum(axis=0)                          # already SW-scaled
        cst = const_all[cols] * scale * SW
        wqka = np.concatenate([u, cst])[None, :].astype(nbf)
        wv = (wg[:, vcols] * SW).astype(nbf)
        uv = (wg[:, vcols].sum(axis=0) * SW)[None, :].astype(nbf)
        # proj weights for this head group: [8dc, 2pp, 64, 256] SwI head pairs
        wpg = _CACHE["w_proj_rows"][g]                    # [4h, 64, 1024] f32
        wp = np.empty((8, 2, 64, 256), nf8)
        for dc in range(8):
            for pp in range(2):
                A = wpg[2 * pp, :, dc * 128:(dc + 1) * 128] * SW
                Bm = wpg[2 * pp + 1, :, dc * 128:(dc + 1) * 128] * SW
                wp[dc, pp] = _swi(A, Bm).astype(nf8)
        in_maps.append({
            "x8": xT_f8[b], "xT": xT_bf[b], "wqk": wqk, "wqka": wqka,
            "wv": wv, "uv": uv, "wp": wp, "masks": masks,
        })
    return in_maps


def prep_l2_inputs(hp_flat, ln2_g, ln2_b, w_fc, b_fc, w_fc2, b_fc2):
    g2 = np.asarray(ln2_g, np.float32)
    b2 = np.asarray(ln2_b, np.float32)
    wfc = np.asarray(w_fc, np.float32)
    bfc = np.asarray(b_fc, np.float32)
    w2 = np.asarray(w_fc2, np.float32)
    b22 = np.asarray(b_fc2, np.float32)

    wfc_g = (wfc * g2[:, None]).astype(nbf)
    wfc_t = np.ascontiguousarray(
        wfc_g.reshape(8, 128, 32, 128).transpose(2, 0, 1, 3))  # [32ct, 8ci, 128, 128]
    cfc = (b2 @ wfc + bfc).astype(np.float32).reshape(32, 128).T.copy()  # [128, 32]
    w2_t = np.ascontiguousarray(w2.astype(nbf).reshape(32, 128, 1024))
    b2r = np.broadcast_to(b22.astype(np.float32), (128, 1024)).copy()

    in_maps = []
    for c in range(NCORES):
        rows = slice(c * ROWS2, (c + 1) * ROWS2)
        hT = np.ascontiguousarray(hp_flat[rows].T).astype(nbf)
        in_maps.append({
            "hT": hT, "wfc": wfc_t, "cfc": cfc,
            "w2": w2_t, "b2r": b2r,
        })
    return in_maps


def combine_l1(hidden_states, parts, b_attn, ln1_b, w_attn, b_proj):
    h = np.asarray(hidden_states, np.float32)
    wa = np.asarray(w_attn, np.float32)
    const_v = (np.asarray(ln1_b, np.float32) @ wa[:, 2 * D:]
               + np.asarray(b_attn, np.float32)[2 * D:])
    y_const = const_v @ np.asarray(_CACHE["w_proj_full"], np.float32) \
        + np.asarray(b_proj, np.float32)
    hp = h.copy()
    for b in range(B):
        acc = np.zeros((D, S), np.float32)
        for g in range(HG):
            acc += parts[b * HG + g]
        hp[b] += acc.T * ISW + y_const[None, :]
    return hp


def kernel(hidden_states, ln1_g, ln1_b, w_attn, b_attn, w_proj, b_proj,
           ln2_g, ln2_b, w_fc, b_fc, w_fc2, b_fc2):
    wpj = np.asarray(w_proj, np.float32)
    _CACHE["w_proj_full"] = wpj
    _CACHE["w_proj_rows"] = [
        np.stack([wpj[DH * (HG * g + i):DH * (HG * g + i) + DH, :]
                  for i in range(HG)], axis=0)
        for g in range(HG)
    ]

    nc1 = _get_l1()
    in1 = prep_l1_inputs(hidden_states, ln1_g, ln1_b, w_attn, b_attn)
    res1 = run_bass_kernel_spmd(nc1, in1, list(range(NCORES)))
    parts = [res1.results[c]["out"] for c in range(NCORES)]

    hp = combine_l1(hidden_states, parts, b_attn, ln1_b, w_attn, b_proj)
    hp_flat = hp.reshape(B * S, D)

    nc2 = _get_l2()
    in2 = prep_l2_inputs(hp_flat, ln2_g, ln2_b, w_fc, b_fc, w_fc2, b_fc2)
    res2 = run_bass_kernel_spmd(nc2, in2, list(range(NCORES)))

    out = hp_flat.copy()
    for c in range(NCORES):
        out[c * ROWS2:(c + 1) * ROWS2] += res2.results[c]["out"]
    return out.reshape(B, S, D).astype(np.float32)
